# revision 29
# baseline (speedup 1.0000x reference)
"""Fused transformer block (QKV -> diag-zeroed attention -> FFN -> LayerNorm)
for Trainium2, sharded over 8 NeuronCores.

Sharding: each core owns one (batch, 512-query-block) pair: core c handles
batch c//4, queries [(c%4)*512, (c%4)*512+512). No collectives: each core
redundantly computes full-batch K/V projections, then attention for its
queries over all 16 heads, then the FFN + LayerNorm for its 512 tokens.

Per-core layout trick: the host sends x[b] rolled by -q0 along tokens and
transposed (xT [D, S]), so the query block is always tokens [0, 512) and the
attention diagonal always sits in k-tiles 0..3 at a fixed position --
the compiled program is identical across cores (true SPMD), only data differs.

Attention math (diagonal zeroed *after* softmax, per reference):
  eT[k, q] = exp(score[k, q] / 32)                (scores are small: no max-sub)
  num[d, q], denom[q] = (V | ones)^T @ eT         (ones-augmented V matmul)
  out[d, q] = (num[d,q] - eT[q,q] * V[q,d]) / denom[q]
GELU uses the ScalarEngine's exact (erf-flavor) Gelu LUT.
"""

import os
import numpy as np
import ml_dtypes
from contextlib import ExitStack

import concourse.bass as bass
import concourse.mybir as mybir
import concourse.tile as tile
from concourse import bacc
from concourse.bass_utils import run_bass_kernel_spmd

AF = mybir.ActivationFunctionType
ALU = mybir.AluOpType
BF16 = mybir.dt.bfloat16
F32 = mybir.dt.float32
F32R = mybir.dt.float32r

N_CORES = 8
B, S, D, H, HD, F = 2, 2048, 1024, 16, 64, 4096
QB = 512          # queries per core
HP = H // 2       # 8 head pairs
CT = D // 128     # 8 contraction tiles over D
TT = S // 512     # 4 token 512-blocks
NKT = S // 128    # 16 key 128-tiles
FT = F // 128     # 32 f 128-tiles
INV_SQRT_D = 1.0 / 32.0
LN_EPS = 1e-5

_NC = None


def _r(ap, pattern, **kw):
    return ap.rearrange(pattern, **kw)


def _emit(tc, nc, io):
    KPH = os.environ.get("KPH", "full")
    ts = bass.ts

    def bisect_out(pool, tiles, n=4):
        # debug aid: route intermediate tiles to y so a phase-truncated
        # kernel still has live outputs
        for i in range(n):
            st = pool.tile([128, 512], F32, tag="bis", name="bis")
            nc.vector.tensor_copy(st[:], tiles[i][:, 0:512])
            nc.sync.dma_start(io["y"][ts(i, 128), 0:512], st[:])
    with ExitStack() as ctx:
        # ---------------- constants resident for the whole kernel ----------
        cpool = ctx.enter_context(tc.tile_pool(name="consts", bufs=1))
        eye_sb = cpool.tile([128, 128], BF16)
        nc.sync.dma_start(eye_sb[:], io["eye"][:])
        bq_sb = cpool.tile([128, HP], F32)
        nc.sync.dma_start(bq_sb[:], io["bq_r"][:])
        bk_sb = cpool.tile([128, HP], F32)
        nc.sync.dma_start(bk_sb[:], io["bk_r"][:])
        bvp_sb = cpool.tile([128, HP], F32)
        nc.sync.dma_start(bvp_sb[:], io["bv_r"][:])
        b1_sb = cpool.tile([128, FT], F32)
        nc.sync.dma_start(b1_sb[:], io["b1_r"][:])
        eps_sb = cpool.tile([128, 1], F32)
        nc.vector.memset(eps_sb[:], LN_EPS)

        # ---------------- attention-lifetime activations -------------------
        outtp = ctx.enter_context(tc.tile_pool(name="outtp", bufs=HP))
        with ExitStack() as actx:
            ktp = actx.enter_context(tc.tile_pool(name="ktp", bufs=HP))
            vp = actx.enter_context(tc.tile_pool(name="vp", bufs=NKT))
            qtp = actx.enter_context(tc.tile_pool(name="qtp", bufs=HP))
            vtp = actx.enter_context(tc.tile_pool(name="vtp", bufs=HP))
            kt_sb = [ktp.tile([128, S], BF16, tag="kt", name="kt") for _ in range(HP)]
            v_sb = [vp.tile([128, H * (HD + 1)], BF16, tag="v", name="v") for _ in range(NKT)]
            qt_sb = [qtp.tile([128, QB], BF16, tag="qt", name="qt") for _ in range(HP)]
            vt_sb = [vtp.tile([128, QB], BF16, tag="vt", name="vt") for _ in range(HP)]
            outt_sb = [outtp.tile([128, QB], BF16, tag="outt", name="outt") for _ in range(HP)]

            # ---------- phases A-D: projections (xt/weights streamed) ------
            with ExitStack() as lctx:
                xtp = lctx.enter_context(tc.tile_pool(name="xtp", bufs=CT))
                wp = lctx.enter_context(tc.tile_pool(name="wp", bufs=CT))
                bvbcp = lctx.enter_context(tc.tile_pool(name="bvbc", bufs=1))
                pps = lctx.enter_context(
                    tc.tile_pool(name="pps", bufs=4, space="PSUM"))

                bvbc_sb = bvbcp.tile([128, D], F32)
                nc.sync.dma_start(bvbc_sb[:], io["bv_bc"][:])

                xt_sb = []
                for c in range(CT):
                    t = xtp.tile([128, S], BF16, tag="xt", name="xt")
                    nc.sync.dma_start(t[:], io["xt"][ts(c, 128), :])
                    xt_sb.append(t)

                # K^T for all tokens: kt_sb[hp][d, k] (+bk), head pair hp
                wk_sb = []
                for c in range(CT):
                    t = wp.tile([128, D], BF16, tag="w", name="w")
                    nc.sync.dma_start(t[:], io["wk"][ts(c, 128), :])
                    wk_sb.append(t)
                for tt in range(TT):
                    for hp in range(HP):
                        ps = pps.tile([128, 512], F32, tag="pp", name="pp")
                        for c in range(CT):
                            nc.tensor.matmul(
                                ps[:], wk_sb[c][:, ts(hp, 128)],
                                xt_sb[c][:, ts(tt, 512)],
                                start=(c == 0), stop=(c == CT - 1))
                        nc.vector.tensor_scalar_add(
                            kt_sb[hp][:, ts(tt, 512)], ps[:],
                            bk_sb[:, hp:hp + 1])

                # Q^T for the query block (rolled tokens 0..QB)
                wq_sb = []
                for c in range(CT):
                    t = wp.tile([128, D], BF16, tag="w", name="w")
                    nc.sync.dma_start(t[:], io["wq"][ts(c, 128), :])
                    wq_sb.append(t)
                for hp in range(HP):
                    ps = pps.tile([128, 512], F32, tag="pp", name="pp")
                    for c in range(CT):
                        nc.tensor.matmul(
                            ps[:], wq_sb[c][:, ts(hp, 128)],
                            xt_sb[c][:, 0:QB],
                            start=(c == 0), stop=(c == CT - 1))
                    nc.vector.tensor_scalar_add(
                        qt_sb[hp][:], ps[:], bq_sb[:, hp:hp + 1])

                # V^T for the query block (for the diagonal correction)
                wv_sb = []
                for c in range(CT):
                    t = wp.tile([128, D], BF16, tag="w", name="w")
                    nc.sync.dma_start(t[:], io["wv"][ts(c, 128), :])
                    wv_sb.append(t)
                for hp in range(HP):
                    ps = pps.tile([128, 512], F32, tag="pp", name="pp")
                    for c in range(CT):
                        nc.tensor.matmul(
                            ps[:], wv_sb[c][:, ts(hp, 128)],
                            xt_sb[c][:, 0:QB],
                            start=(c == 0), stop=(c == CT - 1))
                    nc.vector.tensor_scalar_add(
                        vt_sb[hp][:], ps[:], bvp_sb[:, hp:hp + 1])

                # V (token-major, 65-wide per head: last column = 1.0)
                for mt in range(NKT):
                    vtile = v_sb[mt]
                    v3 = _r(vtile, "p (h e) -> p h e", e=HD + 1)
                    nc.vector.memset(v3[:, :, HD:HD + 1], 1.0)
                    for nh in range(2):
                        ps = pps.tile([128, 512], F32, tag="pp", name="pp")
                        for c in range(CT):
                            nc.tensor.matmul(
                                ps[:], xt_sb[c][:, ts(mt, 128)],
                                wv_sb[c][:, ts(nh, 512)],
                                start=(c == 0), stop=(c == CT - 1))
                        nc.vector.scalar_tensor_tensor(
                            v3[:, ts(nh, 8), 0:HD],
                            _r(ps, "p (h e) -> p h e", e=HD),
                            1.0,
                            _r(bvbc_sb[:, ts(nh, 512)], "p (h e) -> p h e", e=HD),
                            op0=ALU.mult, op1=ALU.add)

            if KPH == "ad":
                bp = actx.enter_context(tc.tile_pool(name="bp", bufs=2))
                bisect_out(bp, qt_sb)
                return
            # ---------------- phase E: attention ---------------------------
            eps_ = actx.enter_context(
                tc.tile_pool(name="spsum", bufs=2, space="PSUM"))
            ops_ = actx.enter_context(
                tc.tile_pool(name="opsum", bufs=4, space="PSUM"))
            etp = actx.enter_context(tc.tile_pool(name="etp", bufs=3))
            scrp = actx.enter_context(tc.tile_pool(name="scrp", bufs=2))
            dcp = actx.enter_context(tc.tile_pool(name="dcp", bufs=2))
            bcp = actx.enter_context(tc.tile_pool(name="bcp", bufs=4))
            tmpp = actx.enter_context(tc.tile_pool(name="tmpp", bufs=6))
            drp = actx.enter_context(
                tc.tile_pool(name="drp", bufs=8, space="DRAM"))

            for hp in range(HP):
                out_ps = [ops_.tile([128, 512], F32, tag="ops", name="ops")
                          for _ in range(2)]
                dcol_f = dcp.tile([128, 8], F32, tag="dcf", name="dcf")
                for kt in range(NKT):
                    s_ps = eps_.tile([128, 1024], F32, tag="sps", name="sps")
                    for half in range(2):
                        nc.tensor.matmul(
                            s_ps[:, ts(half, 512)],
                            kt_sb[hp][ts(half, 64), ts(kt, 128)],
                            qt_sb[hp][ts(half, 64), :],
                            start=True, stop=True)
                    et = etp.tile([128, 1024], BF16, tag="et", name="et")
                    nc.scalar.activation(et[:], s_ps[:], AF.Exp,
                                         scale=INV_SQRT_D)
                    for half in range(2):
                        h = hp * 2 + half
                        nc.tensor.matmul(
                            out_ps[half][0:HD + 1, :],
                            v_sb[kt][:, h * (HD + 1):(h + 1) * (HD + 1)],
                            et[:, ts(half, 512)],
                            start=(kt == 0), stop=(kt == NKT - 1))
                        if kt < QB // 128 and KPH != "attn1":
                            junk = scrp.tile([128, 128], BF16, tag="junk", name="junk")
                            nc.vector.tensor_mul(
                                junk[:],
                                et[:, half * 512 + kt * 128:
                                   half * 512 + (kt + 1) * 128],
                                eye_sb[:])
                            nc.vector.reduce_sum(
                                dcol_f[:, half * 4 + kt:half * 4 + kt + 1],
                                junk[:], axis=mybir.AxisListType.X)
                if KPH in ("attn1", "attn2"):
                    for half in range(2):
                        evac = tmpp.tile([65, 512], F32, tag="evac",
                                         name="evac")
                        nc.vector.tensor_copy(evac[:],
                                              out_ps[half][0:HD + 1, :])
                        nc.vector.tensor_copy(outt_sb[hp][0:HD, :],
                                              evac[0:HD, :])
                    continue
                # assemble num [128, 512], denom/ediag broadcast tiles, then
                # do the whole correction as full-128-partition DVE ops
                num_sb = tmpp.tile([128, 512], F32, tag="num", name="num")
                bc_de = bcp.tile([128, 512], F32, tag="bc", name="bc")
                bc_ed = bcp.tile([128, 512], F32, tag="bc", name="bc")
                for half in range(2):
                    # diag columns -> DRAM (transposed to a row)
                    ed_dram = drp.tile([1, 512], F32, tag="edd", name="edd")
                    nc.sync.dma_start(
                        ed_dram.rearrange("o (j p) -> o p j", p=128),
                        dcol_f[:, half * 4:half * 4 + 4])
                    # evacuate this half's psum (num rows + denom row)
                    evac = tmpp.tile([65, 512], F32, tag="evac", name="evac")
                    nc.vector.tensor_copy(evac[:], out_ps[half][0:HD + 1, :])
                    de_dram = drp.tile([1, 512], F32, tag="ded", name="ded")
                    nc.sync.dma_start(de_dram[:], evac[HD:HD + 1, :])
                    nc.sync.dma_start(num_sb[ts(half, 64), :],
                                      evac[0:HD, :])
                    nc.sync.dma_start(
                        bc_de[ts(half, 64), :],
                        de_dram[0:1, :].to_broadcast((64, 512)))
                    nc.sync.dma_start(
                        bc_ed[ts(half, 64), :],
                        ed_dram[0:1, :].to_broadcast((64, 512)))
                rcp = tmpp.tile([128, 512], F32, tag="num", name="rcp")
                nc.vector.reciprocal(rcp[:], bc_de[:])
                t1 = tmpp.tile([128, 512], F32, tag="num", name="t1")
                nc.vector.tensor_mul(t1[:], vt_sb[hp][:], bc_ed[:])
                t2 = tmpp.tile([128, 512], F32, tag="num", name="t2")
                nc.vector.tensor_sub(t2[:], num_sb[:], t1[:])
                nc.vector.tensor_mul(outt_sb[hp][:], t2[:], rcp[:])

        if KPH.startswith("attn"):
            bp2 = ctx.enter_context(tc.tile_pool(name="bp2", bufs=2))
            bisect_out(bp2, outt_sb)
            return
        # ---------------- phase F: FFN1 + exact GELU -----------------------
        h1p = ctx.enter_context(tc.tile_pool(name="h1p", bufs=FT))
        h1_sb = [h1p.tile([128, QB], BF16, tag="h1", name="h1") for _ in range(FT)]
        with ExitStack() as fctx:
            w1p = fctx.enter_context(tc.tile_pool(name="w1p", bufs=CT))
            fps = fctx.enter_context(
                tc.tile_pool(name="fpsum", bufs=4, space="PSUM"))
            w1_sb = []
            for c in range(CT):
                t = w1p.tile([128, F], BF16, tag="w1", name="w1")
                nc.sync.dma_start(t[:], io["w1"][ts(c, 128), :])
                w1_sb.append(t)
            for ft in range(FT):
                ps = fps.tile([128, 512], F32, tag="fp", name="fp")
                for c in range(CT):
                    nc.tensor.matmul(ps[:], w1_sb[c][:, ts(ft, 128)],
                                     outt_sb[c][:],
                                     start=(c == 0), stop=(c == CT - 1))
                nc.scalar.activation(h1_sb[ft][:], ps[:], AF.Gelu,
                                     bias=b1_sb[:, ft:ft + 1])

        if KPH == "ffn1":
            bp3 = ctx.enter_context(tc.tile_pool(name="bp3", bufs=2))
            bisect_out(bp3, h1_sb)
            return
        # ---------------- phase G: FFN2 + LayerNorm ------------------------
        with ExitStack() as gctx:
            lcp = gctx.enter_context(tc.tile_pool(name="lcp", bufs=1))
            w2p = gctx.enter_context(tc.tile_pool(name="w2p", bufs=FT))
            gps = gctx.enter_context(
                tc.tile_pool(name="gpsum", bufs=2, space="PSUM"))
            lnp = gctx.enter_context(tc.tile_pool(name="lnp", bufs=2))
            stp = gctx.enter_context(tc.tile_pool(name="stp", bufs=4))

            b2bc_sb = lcp.tile([128, D], F32)
            nc.sync.dma_start(b2bc_sb[:], io["b2_bc"][:])
            g_sb = lcp.tile([128, D], F32)
            nc.sync.dma_start(g_sb[:], io["gamma_bc"][:])
            be_sb = lcp.tile([128, D], F32)
            nc.sync.dma_start(be_sb[:], io["beta_bc"][:])

            w2_sb = []
            for ft in range(FT):
                t = w2p.tile([128, D], BF16, tag="w2", name="w2")
                nc.sync.dma_start(t[:], io["w2"][ts(ft, 128), :])
                w2_sb.append(t)

            for mt in range(QB // 128):
                ps = gps.tile([128, 1024], F32, tag="gp", name="gp")
                for nh in range(2):
                    for ft in range(FT):
                        nc.tensor.matmul(
                            ps[:, ts(nh, 512)],
                            h1_sb[ft][:, ts(mt, 128)],
                            w2_sb[ft][:, ts(nh, 512)],
                            start=(ft == 0), stop=(ft == FT - 1))
                h2 = lnp.tile([128, D], F32, tag="h2", name="h2")
                nc.vector.tensor_add(h2[:], ps[:], b2bc_sb[:])
                mu = stp.tile([128, 1], F32, tag="st", name="st")
                nc.vector.reduce_sum(mu[:], h2[:], axis=mybir.AxisListType.X)
                mneg = stp.tile([128, 1], F32, tag="st", name="st")
                nc.scalar.mul(mneg[:], mu[:], -1.0 / D)
                hc = lnp.tile([128, D], F32, tag="h2", name="h2")
                nc.vector.tensor_scalar_add(hc[:], h2[:], mneg[:])
                sq = lnp.tile([128, D], BF16, tag="sq", name="sq")
                ssq = stp.tile([128, 1], F32, tag="st", name="st")
                nc.scalar.activation(sq[:], hc[:], AF.Square,
                                     accum_out=ssq[:])
                std = stp.tile([128, 1], F32, tag="st", name="st")
                nc.scalar.activation(std[:], ssq[:], AF.Sqrt,
                                     scale=1.0 / D, bias=eps_sb[:])
                rstd = stp.tile([128, 1], F32, tag="st", name="st")
                nc.vector.reciprocal(rstd[:], std[:])
                yn = lnp.tile([128, D], F32, tag="h2", name="h2")
                nc.vector.scalar_tensor_tensor(
                    yn[:], hc[:], rstd[:], g_sb[:],
                    op0=ALU.mult, op1=ALU.mult)
                yf = lnp.tile([128, D], F32, tag="h2", name="h2")
                nc.vector.tensor_add(yf[:], yn[:], be_sb[:])
                nc.sync.dma_start(io["y"][ts(mt, 128), :], yf[:])


def _build():
    nc = bacc.Bacc("TRN2", target_bir_lowering=False, debug=False,
                   num_devices=N_CORES)
    io = {}

    def inp(name, shape, dt):
        io[name] = nc.dram_tensor(name, shape, dt, kind="ExternalInput").ap()

    inp("xt", [D, S], BF16)
    inp("wq", [D, D], BF16)
    inp("wk", [D, D], BF16)
    inp("wv", [D, D], BF16)
    inp("w1", [D, F], BF16)
    inp("w2", [F, D], BF16)
    inp("bq_r", [128, HP], F32)
    inp("bk_r", [128, HP], F32)
    inp("bv_r", [128, HP], F32)
    inp("b1_r", [128, FT], F32)
    inp("bv_bc", [128, D], F32)
    inp("b2_bc", [128, D], F32)
    inp("gamma_bc", [128, D], F32)
    inp("beta_bc", [128, D], F32)
    inp("eye", [128, 128], BF16)
    io["y"] = nc.dram_tensor("y", [QB, D], F32, kind="ExternalOutput").ap()

    with tile.TileContext(nc) as tc:
        _emit(tc, nc, io)
    nc.compile()
    return nc


def _get_nc():
    global _NC
    if _NC is None:
        _NC = _build()
    return _NC


def _prep_maps(x, Wq, bq, Wk, bk, Wv, bv, W1, b1, W2, b2, gamma, beta):
    bf = ml_dtypes.bfloat16
    f4 = np.float32

    def bc(v):
        return np.ascontiguousarray(
            np.broadcast_to(np.asarray(v, f4), (128, D)))

    shared = {
        "wq": np.asarray(Wq, f4).astype(bf),
        "wk": np.asarray(Wk, f4).astype(bf),
        "wv": np.asarray(Wv, f4).astype(bf),
        "w1": np.asarray(W1, f4).astype(bf),
        "w2": np.asarray(W2, f4).astype(bf),
        "bq_r": np.ascontiguousarray(np.asarray(bq, f4).reshape(HP, 128).T),
        "bk_r": np.ascontiguousarray(np.asarray(bk, f4).reshape(HP, 128).T),
        "bv_r": np.ascontiguousarray(np.asarray(bv, f4).reshape(HP, 128).T),
        "b1_r": np.ascontiguousarray(np.asarray(b1, f4).reshape(FT, 128).T),
        "bv_bc": bc(bv),
        "b2_bc": bc(b2),
        "gamma_bc": bc(gamma),
        "beta_bc": bc(beta),
        "eye": np.eye(128, dtype=bf),
    }
    in_maps = []
    for c in range(N_CORES):
        b, q0 = c // (N_CORES // B), (c % (N_CORES // B)) * QB
        xb = np.asarray(x[b], f4)
        xt = np.ascontiguousarray(np.roll(xb, -q0, axis=0).T).astype(bf)
        in_maps.append({**shared, "xt": xt})
    return in_maps


def run_full(inputs, trace=False):
    nc = _get_nc()
    in_maps = _prep_maps(**inputs)
    res = run_bass_kernel_spmd(nc, in_maps, core_ids=list(range(N_CORES)),
                               trace=trace)
    y = np.empty((B, S, D), np.float32)
    for c in range(N_CORES):
        b, q0 = c // (N_CORES // B), (c % (N_CORES // B)) * QB
        y[b, q0:q0 + QB, :] = res.results[c]["y"]
    return y, res


def kernel(**inputs):
    y, _ = run_full(inputs, trace=False)
    return y


# revision 30
# speedup vs baseline: 222.3822x; 222.3822x over previous
"""Fused transformer block (QKV -> diag-zeroed attention -> FFN -> LayerNorm)
for Trainium2, head-sharded over 8 NeuronCores with an AllToAll.

Sharding: core c owns head pair c (heads 2c, 2c+1) for attention over ALL
tokens of both batches -- no redundant K/V work. The attention outputs are
exchanged with a single AllToAll so core c then owns token block c
(batch c//4, queries [(c%4)*512, ...+512)) with the full model dim, and runs
the FFN + LayerNorm for those tokens.

Attention math (diagonal zeroed *after* softmax, per reference):
  eT[k, q] = exp(score[k, q] / 32)             (scores are small: no max-sub)
  num[d, q], denom[q] = (V | ones)^T @ eT      (ones-augmented V matmul)
  out[d, q] = (num[d,q] - eT[q,q] * V[q,d]) / denom[q]

Precision: projections and attention internals in bf16 (errors there are
attenuated by softmax averaging); the FFN path (attention out, W1, h1, W2)
in fp32 with float32r matmuls, since LayerNorm renormalizes the small FFN
signal and any relative error there lands directly on the output.
"""

import os
import numpy as np
import ml_dtypes
from contextlib import ExitStack

import concourse.bass as bass
import concourse.mybir as mybir
import concourse.tile as tile
from concourse import bacc
from concourse.bass_utils import run_bass_kernel_spmd

AF = mybir.ActivationFunctionType
ALU = mybir.AluOpType
BF16 = mybir.dt.bfloat16
F32 = mybir.dt.float32
F32R = mybir.dt.float32r

N_CORES = 8
B, S, D, H, HD, F = 2, 2048, 1024, 16, 64, 4096
QB = 512          # tokens per core after the exchange
CT = D // 128     # 8 contraction tiles over D
TT = S // 512     # 4 token 512-blocks per batch
NKT = S // 128    # 16 key 128-tiles per batch
FT = F // 128     # 32 f 128-tiles
INV_SQRT_D = 1.0 / 32.0
LN_EPS = 1e-5
VW = HD + 1       # 65: V columns per head incl. the ones column

_NC = None


def _r(ap, pattern, **kw):
    return ap.rearrange(pattern, **kw)


def _emit(tc, nc, io):
    KPH = os.environ.get("BASS_KERNEL_BISECT", "full")
    ts = bass.ts

    def bisect_out(pool, tiles, n=4):
        for i in range(n):
            st = pool.tile([128, 512], F32, tag="bis", name="bis")
            nc.vector.tensor_copy(st[:], tiles[i][:, 0:512])
            nc.sync.dma_start(io["y"][ts(i, 128), 0:512], st[:])
    with ExitStack() as ctx:
        # ---------------- constants ----------------------------------------
        cpool = ctx.enter_context(tc.tile_pool(name="consts", bufs=1))
        eye_sb = cpool.tile([128, 128], BF16)
        nc.sync.dma_start(eye_sb[:], io["eye"][:])
        bq_sb = cpool.tile([128, 1], F32)
        nc.sync.dma_start(bq_sb[:], io["bq_hp"][:])
        bk_sb = cpool.tile([128, 1], F32)
        nc.sync.dma_start(bk_sb[:], io["bk_hp"][:])
        bv_sb = cpool.tile([128, 1], F32)
        nc.sync.dma_start(bv_sb[:], io["bv_hp"][:])
        bvbc_sb = cpool.tile([128, 128], F32)
        nc.sync.dma_start(bvbc_sb[:], io["bv_bc2"][:])
        b1_sb = cpool.tile([128, FT], F32)
        nc.sync.dma_start(b1_sb[:], io["b1_r"][:])
        eps_sb = cpool.tile([128, 1], F32)
        nc.vector.memset(eps_sb[:], LN_EPS)

        # outt: token-major attention output after the exchange (fp32)
        outtp = ctx.enter_context(tc.tile_pool(name="outtp", bufs=CT))
        outt_sb = [outtp.tile([128, QB], F32R, tag="outt", name="outt")
                   for _ in range(CT)]

        a2a_in = nc.dram_tensor("a2a_in", [N_CORES, 128, QB], BF16).ap()
        a2a_out = nc.dram_tensor("a2a_out", [N_CORES, 128, QB], BF16).ap()

        with ExitStack() as actx:
            ktp = actx.enter_context(tc.tile_pool(name="ktp", bufs=2))
            qtp = actx.enter_context(tc.tile_pool(name="qtp", bufs=2))
            vtp = actx.enter_context(tc.tile_pool(name="vtp", bufs=2))
            vp = actx.enter_context(tc.tile_pool(name="vp", bufs=2 * NKT))
            kt_sb = [ktp.tile([128, S], BF16, tag="kt", name="kt")
                     for _ in range(B)]
            qt_sb = [qtp.tile([128, S], BF16, tag="qt", name="qt")
                     for _ in range(B)]
            vt_sb = [vtp.tile([128, S], BF16, tag="vt", name="vt")
                     for _ in range(B)]
            v_sb = [[vp.tile([128, 2 * VW], BF16, tag="v", name="v")
                     for _ in range(NKT)] for _ in range(B)]

            # ---------- projections ----------------------------------------
            with ExitStack() as lctx:
                xtp = lctx.enter_context(tc.tile_pool(name="xtp", bufs=CT))
                wp = lctx.enter_context(tc.tile_pool(name="wp", bufs=3 * CT))
                pps = lctx.enter_context(
                    tc.tile_pool(name="pps", bufs=4, space="PSUM"))

                w_sb = {}
                for wname in ("wk", "wq", "wv"):
                    w_sb[wname] = []
                    for c in range(CT):
                        t = wp.tile([128, 128], BF16, tag="w", name="w")
                        nc.sync.dma_start(t[:],
                                          io[wname + "_hp"][ts(c, 128), :])
                        w_sb[wname].append(t)

                for b in range(B):
                    # stream this batch's x^T (the two batches share slots)
                    xt_b = []
                    for c in range(CT):
                        t = xtp.tile([128, S], BF16, tag="xt", name="xt")
                        nc.sync.dma_start(t[:], io[f"xt{b}"][ts(c, 128), :])
                        xt_b.append(t)
                    for wname, dst, bias in (("wk", kt_sb, bk_sb),
                                             ("wq", qt_sb, bq_sb),
                                             ("wv", vt_sb, bv_sb)):
                        for tt in range(TT):
                            ps = pps.tile([128, 512], F32, tag="pp",
                                          name="pp")
                            for c in range(CT):
                                nc.tensor.matmul(
                                    ps[:], w_sb[wname][c][:],
                                    xt_b[c][:, ts(tt, 512)],
                                    start=(c == 0), stop=(c == CT - 1))
                            nc.vector.tensor_scalar_add(
                                dst[b][:, ts(tt, 512)], ps[:], bias[:])

                    # V (token-major, VW-wide per head: last column = 1.0)
                    for mt in range(NKT):
                        vtile = v_sb[b][mt]
                        v3 = _r(vtile, "p (h e) -> p h e", e=VW)
                        nc.vector.memset(v3[:, :, HD:HD + 1], 1.0)
                        ps = pps.tile([128, 512], F32, tag="pp", name="pp")
                        for c in range(CT):
                            nc.tensor.matmul(
                                ps[:, 0:128], xt_b[c][:, ts(mt, 128)],
                                w_sb["wv"][c][:],
                                start=(c == 0), stop=(c == CT - 1))
                        nc.vector.scalar_tensor_tensor(
                            v3[:, :, 0:HD],
                            _r(ps[:, 0:128], "p (h e) -> p h e", e=HD),
                            1.0,
                            _r(bvbc_sb, "p (h e) -> p h e", e=HD),
                            op0=ALU.mult, op1=ALU.add)

            if KPH == "ad":
                bp = actx.enter_context(tc.tile_pool(name="bp", bufs=2))
                bisect_out(bp, kt_sb + qt_sb, n=4)
                return
            # ---------- attention per (batch, query block) ------------------
            eps_ = actx.enter_context(
                tc.tile_pool(name="spsum", bufs=2, space="PSUM"))
            ops_ = actx.enter_context(
                tc.tile_pool(name="opsum", bufs=4, space="PSUM"))
            etp = actx.enter_context(tc.tile_pool(name="etp", bufs=3))
            scrp = actx.enter_context(tc.tile_pool(name="scrp", bufs=2))
            dcp = actx.enter_context(tc.tile_pool(name="dcp", bufs=2))
            bcp = actx.enter_context(tc.tile_pool(name="bcp", bufs=4))
            tmpp = actx.enter_context(tc.tile_pool(name="tmpp", bufs=6))
            drp = actx.enter_context(
                tc.tile_pool(name="drp", bufs=8, space="DRAM"))

            for b in range(B):
                for qb in range(TT):
                    dest = b * TT + qb
                    out_ps = [ops_.tile([128, 512], F32, tag="ops",
                                        name="ops") for _ in range(2)]
                    dcol_f = dcp.tile([128, 8], F32, tag="dcf", name="dcf")
                    for kt in range(NKT):
                        s_ps = eps_.tile([128, 1024], F32, tag="sps",
                                         name="sps")
                        for half in range(2):
                            nc.tensor.matmul(
                                s_ps[:, ts(half, 512)],
                                kt_sb[b][ts(half, 64), ts(kt, 128)],
                                qt_sb[b][ts(half, 64), ts(qb, 512)],
                                start=True, stop=True)
                        et = etp.tile([128, 1024], BF16, tag="et", name="et")
                        nc.scalar.activation(et[:], s_ps[:], AF.Exp,
                                             scale=INV_SQRT_D)
                        j = kt - qb * 4
                        for half in range(2):
                            nc.tensor.matmul(
                                out_ps[half][0:VW, :],
                                v_sb[b][kt][:, half * VW:(half + 1) * VW],
                                et[:, ts(half, 512)],
                                start=(kt == 0), stop=(kt == NKT - 1))
                            if 0 <= j < 4:
                                junk = scrp.tile([128, 128], BF16,
                                                 tag="junk", name="junk")
                                nc.vector.tensor_mul(
                                    junk[:],
                                    et[:, half * 512 + j * 128:
                                       half * 512 + (j + 1) * 128],
                                    eye_sb[:])
                                nc.vector.reduce_sum(
                                    dcol_f[:, half * 4 + j:half * 4 + j + 1],
                                    junk[:], axis=mybir.AxisListType.X)
                    # correction + normalization, then ship to the exchange
                    num_sb = tmpp.tile([128, 512], F32, tag="num", name="num")
                    bc_de = bcp.tile([128, 512], F32, tag="bc", name="bc")
                    bc_ed = bcp.tile([128, 512], F32, tag="bc", name="bc")
                    for half in range(2):
                        ed_dram = drp.tile([1, 512], F32, tag="edd",
                                           name="edd")
                        nc.sync.dma_start(
                            ed_dram.rearrange("o (j p) -> o p j", p=128),
                            dcol_f[:, half * 4:half * 4 + 4])
                        evac = tmpp.tile([VW, 512], F32, tag="evac",
                                         name="evac")
                        nc.vector.tensor_copy(evac[:],
                                              out_ps[half][0:VW, :])
                        de_dram = drp.tile([1, 512], F32, tag="ded",
                                           name="ded")
                        nc.sync.dma_start(de_dram[:], evac[HD:HD + 1, :])
                        nc.sync.dma_start(num_sb[ts(half, 64), :],
                                          evac[0:HD, :])
                        nc.sync.dma_start(
                            bc_de[ts(half, 64), :],
                            de_dram[0:1, :].to_broadcast((64, 512)))
                        nc.sync.dma_start(
                            bc_ed[ts(half, 64), :],
                            ed_dram[0:1, :].to_broadcast((64, 512)))
                    rcp = tmpp.tile([128, 512], F32, tag="num", name="rcp")
                    nc.vector.reciprocal(rcp[:], bc_de[:])
                    t1 = tmpp.tile([128, 512], F32, tag="num", name="t1")
                    nc.vector.tensor_mul(t1[:],
                                         vt_sb[b][:, ts(qb, 512)], bc_ed[:])
                    t2 = tmpp.tile([128, 512], F32, tag="num", name="t2")
                    nc.vector.tensor_sub(t2[:], num_sb[:], t1[:])
                    outf = tmpp.tile([128, 512], BF16, tag="outf",
                                     name="outf")
                    nc.vector.tensor_mul(outf[:], t2[:], rcp[:])
                    nc.sync.dma_start(a2a_in[dest, :, :], outf[:])

        if KPH == "attn":
            bp2 = ctx.enter_context(tc.tile_pool(name="bp2", bufs=2))
            bisect_out(bp2, outt_sb)
            return
        # ---------------- AllToAll exchange --------------------------------
        nc.gpsimd.collective_compute(
            "AllToAll", ALU.bypass,
            replica_groups=[list(range(N_CORES))],
            ins=[a2a_in[:]], outs=[a2a_out[:]])
        obp = ctx.enter_context(tc.tile_pool(name="obp", bufs=4))
        for i in range(CT):
            ob = obp.tile([128, QB], BF16, tag="ob", name="ob")
            nc.sync.dma_start(ob[:], a2a_out[i, :, :])
            nc.vector.tensor_copy(outt_sb[i][:], ob[:])

        if KPH == "a2a":
            bp3 = ctx.enter_context(tc.tile_pool(name="bp3", bufs=2))
            bisect_out(bp3, outt_sb)
            return
        # ---------------- FFN1 + exact GELU --------------------------------
        h1p = ctx.enter_context(tc.tile_pool(name="h1p", bufs=FT))
        h1_sb = [h1p.tile([128, QB], F32R, tag="h1", name="h1")
                 for _ in range(FT)]
        with ExitStack() as fctx:
            fps = fctx.enter_context(
                tc.tile_pool(name="fpsum", bufs=4, space="PSUM"))
            w1f = fctx.enter_context(tc.tile_pool(name="w1f", bufs=24))
            w1_cur = None
            for ft in range(FT):
                fchunk, fo = divmod(ft, 4)
                if fo == 0:
                    w1_cur = []
                    for c in range(CT):
                        t = w1f.tile([128, 512], F32R, tag="w1", name="w1")
                        nc.sync.dma_start(
                            t[:], io["w1"][ts(c, 128),
                                           fchunk * 512:(fchunk + 1) * 512])
                        w1_cur.append(t)
                ps = fps.tile([128, 512], F32, tag="fp", name="fp")
                for c in range(CT):
                    nc.tensor.matmul(
                        ps[:], w1_cur[c][:, ts(fo, 128)],
                        outt_sb[c][:],
                        start=(c == 0), stop=(c == CT - 1))
                nc.scalar.activation(h1_sb[ft][:], ps[:], AF.Gelu,
                                     bias=b1_sb[:, ft:ft + 1])

        # ---------------- FFN2 + LayerNorm ---------------------------------
        with ExitStack() as gctx:
            lcp = gctx.enter_context(tc.tile_pool(name="lcp", bufs=1))
            w2p = gctx.enter_context(tc.tile_pool(name="w2p", bufs=24))
            gps = gctx.enter_context(
                tc.tile_pool(name="gpsum", bufs=4, space="PSUM"))
            h2p = gctx.enter_context(tc.tile_pool(name="h2p", bufs=4))
            lnp = gctx.enter_context(tc.tile_pool(name="lnp", bufs=2))
            stp = gctx.enter_context(tc.tile_pool(name="stp", bufs=4))

            b2bc_sb = lcp.tile([128, D], F32)
            nc.sync.dma_start(b2bc_sb[:], io["b2_bc"][:])
            g_sb = lcp.tile([128, D], F32)
            nc.sync.dma_start(g_sb[:], io["gamma_bc"][:])
            be_sb = lcp.tile([128, D], F32)
            nc.sync.dma_start(be_sb[:], io["beta_bc"][:])

            h2_sb = [h2p.tile([128, D], F32, tag="h2s", name="h2s")
                     for _ in range(QB // 128)]
            for nh in range(2):
                w2_sb = []
                for ft in range(FT):
                    t = w2p.tile([128, 512], F32R, tag="w2", name="w2")
                    nc.sync.dma_start(t[:],
                                      io["w2"][ts(ft, 128), ts(nh, 512)])
                    w2_sb.append(t)
                for mt in range(QB // 128):
                    ps = gps.tile([128, 512], F32, tag="gp", name="gp")
                    for ft in range(FT):
                        nc.tensor.matmul(
                            ps[:], h1_sb[ft][:, ts(mt, 128)],
                            w2_sb[ft][:],
                            start=(ft == 0), stop=(ft == FT - 1))
                    nc.vector.tensor_add(h2_sb[mt][:, ts(nh, 512)], ps[:],
                                         b2bc_sb[:, ts(nh, 512)])
            for mt in range(QB // 128):
                h2 = h2_sb[mt]
                mu = stp.tile([128, 1], F32, tag="st", name="st")
                nc.vector.reduce_sum(mu[:], h2[:], axis=mybir.AxisListType.X)
                mneg = stp.tile([128, 1], F32, tag="st", name="st")
                nc.scalar.mul(mneg[:], mu[:], -1.0 / D)
                hc = lnp.tile([128, D], F32, tag="ln", name="hc")
                nc.vector.tensor_scalar_add(hc[:], h2[:], mneg[:])
                sq = lnp.tile([128, D], BF16, tag="sq", name="sq")
                ssq = stp.tile([128, 1], F32, tag="st", name="st")
                nc.scalar.activation(sq[:], hc[:], AF.Square,
                                     accum_out=ssq[:])
                std = stp.tile([128, 1], F32, tag="st", name="st")
                nc.scalar.activation(std[:], ssq[:], AF.Sqrt,
                                     scale=1.0 / D, bias=eps_sb[:])
                rstd = stp.tile([128, 1], F32, tag="st", name="st")
                nc.vector.reciprocal(rstd[:], std[:])
                yn = lnp.tile([128, D], F32, tag="ln", name="yn")
                nc.vector.scalar_tensor_tensor(
                    yn[:], hc[:], rstd[:], g_sb[:],
                    op0=ALU.mult, op1=ALU.mult)
                yf = lnp.tile([128, D], F32, tag="ln", name="yf")
                nc.vector.tensor_add(yf[:], yn[:], be_sb[:])
                nc.sync.dma_start(io["y"][ts(mt, 128), :], yf[:])


def _build():
    nc = bacc.Bacc("TRN2", target_bir_lowering=False, debug=False,
                   num_devices=N_CORES)
    io = {}

    def inp(name, shape, dt):
        io[name] = nc.dram_tensor(name, shape, dt, kind="ExternalInput").ap()

    inp("xt0", [D, S], BF16)
    inp("xt1", [D, S], BF16)
    inp("wq_hp", [D, 128], BF16)
    inp("wk_hp", [D, 128], BF16)
    inp("wv_hp", [D, 128], BF16)
    inp("w1", [D, F], F32R)
    inp("w2", [F, D], F32R)
    inp("bq_hp", [128, 1], F32)
    inp("bk_hp", [128, 1], F32)
    inp("bv_hp", [128, 1], F32)
    inp("bv_bc2", [128, 128], F32)
    inp("b1_r", [128, FT], F32)
    inp("b2_bc", [128, D], F32)
    inp("gamma_bc", [128, D], F32)
    inp("beta_bc", [128, D], F32)
    inp("eye", [128, 128], BF16)
    io["y"] = nc.dram_tensor("y", [QB, D], F32, kind="ExternalOutput").ap()

    with tile.TileContext(nc) as tc:
        _emit(tc, nc, io)
    nc.compile()
    return nc


def _get_nc():
    global _NC
    if _NC is None:
        _NC = _build()
    return _NC


def _rtf32(a):
    # round fp32 to tf32-like precision (drop 13 low mantissa bits, RN)
    b = a.view(np.uint32)
    b = (b + 0x1000) & np.uint32(0xFFFFE000)
    return b.view(np.float32)


def _prep_maps(x, Wq, bq, Wk, bk, Wv, bv, W1, b1, W2, b2, gamma, beta):
    bf = ml_dtypes.bfloat16
    f4 = np.float32

    def bc(v, n=D):
        return np.ascontiguousarray(
            np.broadcast_to(np.asarray(v, f4), (128, n)))

    xt0 = np.ascontiguousarray(np.asarray(x[0], f4).T).astype(bf)
    xt1 = np.ascontiguousarray(np.asarray(x[1], f4).T).astype(bf)
    shared = {
        "xt0": xt0, "xt1": xt1,
        "w1": _rtf32(np.ascontiguousarray(np.asarray(W1, f4))),
        "w2": _rtf32(np.ascontiguousarray(np.asarray(W2, f4))),
        "b1_r": np.ascontiguousarray(np.asarray(b1, f4).reshape(FT, 128).T),
        "b2_bc": bc(b2),
        "gamma_bc": bc(gamma),
        "beta_bc": bc(beta),
        "eye": np.eye(128, dtype=bf),
    }
    Wqf, Wkf, Wvf = (np.asarray(w, f4) for w in (Wq, Wk, Wv))
    bqf, bkf, bvf = (np.asarray(v, f4) for v in (bq, bk, bv))
    in_maps = []
    for c in range(N_CORES):
        sl = slice(c * 128, (c + 1) * 128)
        in_maps.append({
            **shared,
            "wq_hp": np.ascontiguousarray(Wqf[:, sl]).astype(bf),
            "wk_hp": np.ascontiguousarray(Wkf[:, sl]).astype(bf),
            "wv_hp": np.ascontiguousarray(Wvf[:, sl]).astype(bf),
            "bq_hp": np.ascontiguousarray(bqf[sl]).reshape(128, 1),
            "bk_hp": np.ascontiguousarray(bkf[sl]).reshape(128, 1),
            "bv_hp": np.ascontiguousarray(bvf[sl]).reshape(128, 1),
            "bv_bc2": bc(bvf[sl], 128),
        })
    return in_maps


def run_full(inputs, trace=False):
    nc = _get_nc()
    in_maps = _prep_maps(**inputs)
    res = run_bass_kernel_spmd(nc, in_maps, core_ids=list(range(N_CORES)),
                               trace=trace)
    y = np.empty((B, S, D), np.float32)
    for c in range(N_CORES):
        b, q0 = c // (N_CORES // B), (c % (N_CORES // B)) * QB
        y[b, q0:q0 + QB, :] = res.results[c]["y"]
    return y, res


def kernel(**inputs):
    y, _ = run_full(inputs, trace=False)
    return y
